# revision 1
# baseline (speedup 1.0000x reference)
"""Trainium2 Bass kernel for CrossMerge3D.

Input ys: [B=2, S=12, C=96, 32, 32, 32] f32. For each (b, c):
  out = (mA + perm_j(mB) + perm_k(mC)) / 12
where, with the 12 scans split into 3 groups of 4, each group combines as
  m_g = s0 + s1 + flip(s2 + s3)   (flip over the flattened 32^3 volume)
and group B's volume is stored as (j,k,i), group C's as (k,i,j).

Sharding: 8 cores = batch (2) x channel quarters (4) -> 24 channels/core.
No cross-core communication.

Per-core layout: 4 channels x 32 leading-spatial -> 128 SBUF partitions,
1024-wide free dim, 6 macro tiles. Scan-pair 1 MiB loads alternate across
both HWDGE rings; store halves split between SWDGE (gpsimd, a third
descriptor path) and the HWDGE rings. Pair sums on DVE cast to bf16
(tolerance is 2e-2; bf16 keeps ~4e-3). flip = free-dim reversal
(pair-sum read APs) + partition-block reversal, fused with the group
combine and the global 1/12 scale as accumulating bf16 matmuls against
1/12-scaled stationaries (wJ = block anti-diagonal, wI = identity).
Groups run C, B, A per tile so the deep C chain (strided copy +
transpose + extra matmul) overlaps later loads; B's leftover
(k,j)->(j,k) permute folds into the final DVE add's read AP. The final
add reads PSUM + bf16 and writes the f32 output tile in 512-wide halves
(each stored immediately) - no scale op. HW: ~119 us (baseline 145 us);
the 37.75+3.15 MB/core at the 4 KiB-descriptor line rate (~152 ns/desc
x 16 SDMA engines ~= 420 GB/s) bounds the stream at ~97 us, plus ~8 us
preamble and ~7 us drain.
"""

import numpy as np

_B, _S, _C, _D = 2, 12, 96, 32
_NCORE = 8
_CL = _C // 4          # 24 channels per core
_G = _CL // 4          # 6 macro tiles of 4 channels (128 partitions)
_F = _D * _D           # 1024

_nc = None


def _build_program():
    from concourse import bacc, tile, mybir

    f32 = mybir.dt.float32
    bf16 = mybir.dt.bfloat16
    nc = bacc.Bacc(
        "TRN2", target_bir_lowering=False, debug=False, num_devices=_NCORE
    )
    ys = nc.dram_tensor("ys", [_S, _CL, _D, _D, _D], f32, kind="ExternalInput")
    out = nc.dram_tensor("out", [_CL, _D, _D, _D], f32, kind="ExternalOutput")
    ysa = ys.ap()
    outa = out.ap()

    with tile.TileContext(nc) as tc:
        with (
            tc.tile_pool(name="const", bufs=1) as cst,
            tc.tile_pool(name="io", bufs=2) as iop,
            tc.tile_pool(name="tmp", bufs=2) as tmp,
            tc.tile_pool(name="ps", bufs=2, space="PSUM") as ps,
        ):
            scale = 1.0 / 12.0
            # stationaries: 32-block anti-diagonal (x 1/12), identity
            # (x 1/12), and an unscaled identity for the pre-scaled tCt.
            wJ = cst.tile([128, 128], bf16, tag="wJ", name="wJ")
            nc.gpsimd.memset(wJ[:], scale)
            for b in range(4):
                nc.gpsimd.affine_select(
                    out=wJ[32 * b:32 * b + 32, :],
                    in_=wJ[32 * b:32 * b + 32, :],
                    compare_op=mybir.AluOpType.is_equal, fill=0.0,
                    base=-(32 * b + 31), pattern=[[1, 128]],
                    channel_multiplier=1,
                )
            wI = cst.tile([128, 128], bf16, tag="wI", name="wI")
            nc.gpsimd.memset(wI[:], scale)
            nc.gpsimd.affine_select(
                out=wI[:], in_=wI[:],
                compare_op=mybir.AluOpType.is_equal, fill=0.0,
                base=0, pattern=[[1, 128]], channel_multiplier=-1,
            )
            wI1 = cst.tile([128, 128], bf16, tag="wI1", name="wI1")
            nc.gpsimd.memset(wI1[:], 1.0)
            nc.gpsimd.affine_select(
                out=wI1[:], in_=wI1[:],
                compare_op=mybir.AluOpType.is_equal, fill=0.0,
                base=0, pattern=[[1, 128]], channel_multiplier=-1,
            )
            wJ1 = cst.tile([128, 128], bf16, tag="wJ1", name="wJ1")
            nc.gpsimd.memset(wJ1[:], 1.0)
            for b in range(4):
                nc.gpsimd.affine_select(
                    out=wJ1[32 * b:32 * b + 32, :],
                    in_=wJ1[32 * b:32 * b + 32, :],
                    compare_op=mybir.AluOpType.is_equal, fill=0.0,
                    base=-(32 * b + 31), pattern=[[1, 128]],
                    channel_multiplier=1,
                )

            for g in range(_G):
                cs = slice(4 * g, 4 * (g + 1))

                def load_pair(s, tag, eng, bufs=2):
                    t = iop.tile([128, 2 * _F], f32, tag=tag, name=tag,
                                 bufs=bufs)
                    src = ysa[s:s + 2, cs].rearrange(
                        "s c i j k -> (c i) s (j k)"
                    )
                    dst = t[:].rearrange("p (s f) -> p s f", s=2)
                    eng.dma_start(out=dst, in_=src)
                    return t

                # rev pair first (feeds the longer matmul chain), fwd
                # second; the two pairs of a group go to different rings.
                rC = load_pair(10, "rC", nc.sync)
                fC = load_pair(8, "fC", nc.scalar)
                rB = load_pair(6, "rB", nc.sync)
                fB = load_pair(4, "fB", nc.scalar)
                rA = load_pair(2, "rA", nc.sync)
                fA = load_pair(0, "fA", nc.scalar)

                def pair_sums(rt, ft, tag):
                    rs = tmp.tile([128, _F], bf16, tag="rs" + tag,
                                  name="rs" + tag)
                    fs = tmp.tile([128, _F], bf16, tag="fs" + tag,
                                  name="fs" + tag)
                    nc.vector.tensor_add(
                        rs[:], rt[:, 0:_F][:, ::-1], rt[:, _F:2 * _F][:, ::-1])
                    nc.vector.tensor_add(fs[:], ft[:, 0:_F], ft[:, _F:2 * _F])
                    return rs, fs

                _H = (slice(0, _F // 2), slice(_F // 2, _F))

                def combine(rs, fs, name, wJx, wIx, mid=None):
                    # psum = wJx @ rs + wIx @ fs (+ optional mid member
                    # sharing wIx). Matmuls issue J,J then I,I across the
                    # two PSUM banks to minimize LDWEIGHTS switches.
                    # B and C share a tag so PSUM fits in 8 banks.
                    tag = "psA" if name == "A" else "psBC"
                    pf = ps.tile([128, _F], f32, tag=tag, name=name)
                    for h in _H:
                        nc.tensor.matmul(pf[:, h], wJx[:], rs[:][:, h],
                                         start=True, stop=False)
                    if mid is not None:
                        # both mid members before the fs members: the PE is
                        # in-order, and fs depends on the last-arriving
                        # load, so nothing may queue behind its matmuls.
                        for h in _H:
                            nc.tensor.matmul(pf[:, h], wIx[:], mid[:][:, h],
                                             start=False, stop=False)
                    for h in _H:
                        nc.tensor.matmul(pf[:, h], wIx[:], fs[:][:, h],
                                         start=False, stop=True)
                    return pf

                # C: (i,j)->(j,i) free permute as a strided READ in the
                # PSUM->SBUF copy (contiguous write), then block transpose
                # puts tCt in (j,k) layout. C runs UNSCALED (wJ1/wI1); the
                # 1/12 lands when wI accumulates tCt into psA, which lets
                # A's whole group share the wI stationary.
                rsC, fsC = pair_sums(rC, fC, "C")
                psC = combine(rsC, fsC, "C", wJ1, wI1)
                tC = tmp.tile([128, _F], bf16, tag="tC", name="tC")
                nc.scalar.copy(
                    tC[:].rearrange("p (a b) -> p a b", a=_D),
                    psC[:].rearrange("p (a b) -> p a b", a=_D).transpose(
                        [0, 2, 1]),
                )

                # B: j<->i 32x32 block transpose; leftover (k,j) free
                # permute is folded into the final add's read AP.
                rsB, fsB = pair_sums(rB, fB, "B")
                psB = combine(rsB, fsB, "B", wJ, wI)
                tB = tmp.tile([128, _F], bf16, tag="tB", name="tB")
                nc.scalar.copy(tB[:], psB[:])

                # transposes issue after all four B/C pair adds so the DVE
                # never stalls mid-chain waiting on the ACT copies.
                tCt = tmp.tile([128, _F], bf16, tag="tCt", name="tCt")
                nc.vector.transpose(tCt[:], tC[:])
                tBt = tmp.tile([128, _F], bf16, tag="tBt", name="tBt")
                nc.vector.transpose(tBt[:], tB[:])

                # A accumulates C's contribution (unscaled tCt picks up
                # its 1/12 from wI here) plus its own scans; A's fwd pair
                # is the last load, feeding only the shallow wI@fsA ->
                # final add path (no LDWEIGHTS switch left in the drain).
                rsA, fsA = pair_sums(rA, fA, "A")
                psA = combine(rsA, fsA, "A", wJ, wI, mid=tCt)

                # out = psA + tBt read as (j,k); f32 write, no scale op.
                # Done in 512-wide halves, each stored immediately, to
                # shorten the end-of-kernel drain. First half stores via
                # SWDGE (keeps HWDGE rings free for loads mid-stream);
                # second half via an HWDGE ring, whose ~0.6us first-byte
                # latency shortens the drain (SWDGE Q7 emission costs
                # ~2.7us per store).
                o = tmp.tile([128, _F], f32, tag="o", name="o")
                ov = o[:].rearrange("p (j k) -> p j k", j=_D)
                pv = psA[:].rearrange("p (j k) -> p j k", j=_D)
                bv = tBt[:].rearrange("p (k j) -> p j k", k=_D)
                od = outa[cs].rearrange("c i j k -> (c i) (j k)")
                for jh, eng in (
                    (slice(0, _D // 2), nc.gpsimd),
                    (slice(_D // 2, _D), nc.scalar if g % 2 else nc.sync),
                ):
                    nc.vector.tensor_add(ov[:, jh], pv[:, jh], bv[:, jh])
                    eng.dma_start(
                        out=od[:, _F // 2 * (jh.start != 0):][:, :_F // 2],
                        in_=o[:, _F // 2 * (jh.start != 0):][:, :_F // 2],
                    )

    nc.compile()
    return nc


def kernel(ys):
    global _nc
    ys = np.ascontiguousarray(ys, dtype=np.float32)
    assert ys.shape == (_B, _S, _C, _D, _D, _D), ys.shape

    if _nc is None:
        _nc = _build_program()

    from concourse.bass_utils import run_bass_kernel_spmd

    in_maps = []
    for r in range(_NCORE):
        b, q = divmod(r, 4)
        shard = np.ascontiguousarray(ys[b, :, q * _CL:(q + 1) * _CL])
        in_maps.append({"ys": shard})

    res = run_bass_kernel_spmd(_nc, in_maps, list(range(_NCORE)))

    out = np.empty((_B, _C, _D, _D, _D), np.float32)
    for r in range(_NCORE):
        b, q = divmod(r, 4)
        out[b, q * _CL:(q + 1) * _CL] = np.asarray(
            res.results[r]["out"]).astype(np.float32)

    if res.exec_time_ns is not None:
        print(f"HW exec time: {res.exec_time_ns} ns")
    return out



# revision 4
# speedup vs baseline: 1.0071x; 1.0071x over previous
"""Trainium2 Bass kernel for CrossMerge3D.

Input ys: [B=2, S=12, C=96, 32, 32, 32] f32. For each (b, c):
  out = (mA + perm_j(mB) + perm_k(mC)) / 12
where, with the 12 scans split into 3 groups of 4, each group combines as
  m_g = s0 + s1 + flip(s2 + s3)   (flip over the flattened 32^3 volume)
and group B's volume is stored as (j,k,i), group C's as (k,i,j).

Sharding: 8 cores = batch (2) x channel quarters (4) -> 24 channels/core.
No cross-core communication.

Per-core layout: 4 channels x 32 leading-spatial -> 128 SBUF partitions,
1024-wide free dim, 6 macro tiles. Scan-pair 1 MiB loads alternate across
both HWDGE rings; store halves split between SWDGE (gpsimd, a third
descriptor path) and the HWDGE rings. Pair sums on DVE cast to bf16
(tolerance is 2e-2; bf16 keeps ~4e-3). flip = free-dim reversal
(pair-sum read APs) + partition-block reversal, fused with the group
combine and the global 1/12 scale as accumulating bf16 matmuls against
1/12-scaled stationaries (wJ = block anti-diagonal, wI = identity).
Groups run C, B, A per tile so the deep C chain (strided copy +
transpose + extra matmul) overlaps later loads; B's leftover
(k,j)->(j,k) permute folds into the final DVE add's read AP. The final
add reads PSUM + bf16 and writes the f32 output tile in 512-wide halves
(each stored immediately) - no scale op. HW: ~119 us (baseline 145 us);
the 37.75+3.15 MB/core at the 4 KiB-descriptor line rate (~152 ns/desc
x 16 SDMA engines ~= 420 GB/s) bounds the stream at ~97 us, plus ~8 us
preamble and ~7 us drain.
"""

import numpy as np

_B, _S, _C, _D = 2, 12, 96, 32
_NCORE = 8
_CL = _C // 4          # 24 channels per core
_G = _CL // 4          # 6 macro tiles of 4 channels (128 partitions)
_F = _D * _D           # 1024

_nc = None


def _build_program():
    from concourse import bacc, tile, mybir

    f32 = mybir.dt.float32
    bf16 = mybir.dt.bfloat16
    nc = bacc.Bacc(
        "TRN2", target_bir_lowering=False, debug=False, num_devices=_NCORE
    )
    ys = nc.dram_tensor("ys", [_S, _CL, _D, _D, _D], f32, kind="ExternalInput")
    out = nc.dram_tensor("out", [_CL, _D, _D, _D], f32, kind="ExternalOutput")
    ysa = ys.ap()
    outa = out.ap()

    with tile.TileContext(nc) as tc:
        with (
            tc.tile_pool(name="const", bufs=1) as cst,
            tc.tile_pool(name="io", bufs=2) as iop,
            tc.tile_pool(name="tmp", bufs=2) as tmp,
            tc.tile_pool(name="ps", bufs=2, space="PSUM") as ps,
        ):
            scale = 1.0 / 12.0
            # stationaries: 32-block anti-diagonal (x 1/12), identity
            # (x 1/12), and an unscaled identity for the pre-scaled tCt.
            wJ = cst.tile([128, 128], bf16, tag="wJ", name="wJ")
            nc.gpsimd.memset(wJ[:], scale)
            for b in range(4):
                nc.gpsimd.affine_select(
                    out=wJ[32 * b:32 * b + 32, :],
                    in_=wJ[32 * b:32 * b + 32, :],
                    compare_op=mybir.AluOpType.is_equal, fill=0.0,
                    base=-(32 * b + 31), pattern=[[1, 128]],
                    channel_multiplier=1,
                )
            wI = cst.tile([128, 128], bf16, tag="wI", name="wI")
            nc.gpsimd.memset(wI[:], scale)
            nc.gpsimd.affine_select(
                out=wI[:], in_=wI[:],
                compare_op=mybir.AluOpType.is_equal, fill=0.0,
                base=0, pattern=[[1, 128]], channel_multiplier=-1,
            )
            wI1 = cst.tile([128, 128], bf16, tag="wI1", name="wI1")
            nc.gpsimd.memset(wI1[:], 1.0)
            nc.gpsimd.affine_select(
                out=wI1[:], in_=wI1[:],
                compare_op=mybir.AluOpType.is_equal, fill=0.0,
                base=0, pattern=[[1, 128]], channel_multiplier=-1,
            )
            wJ1 = cst.tile([128, 128], bf16, tag="wJ1", name="wJ1")
            nc.gpsimd.memset(wJ1[:], 1.0)
            for b in range(4):
                nc.gpsimd.affine_select(
                    out=wJ1[32 * b:32 * b + 32, :],
                    in_=wJ1[32 * b:32 * b + 32, :],
                    compare_op=mybir.AluOpType.is_equal, fill=0.0,
                    base=-(32 * b + 31), pattern=[[1, 128]],
                    channel_multiplier=1,
                )

            for g in range(_G):
                cs = slice(4 * g, 4 * (g + 1))

                def load_pair(s, tag):
                    # one 512 KiB DMA per scan, pair split across the two
                    # HWDGE rings: rings stay byte-balanced all the way to
                    # the drain and both pair members land together.
                    ts = []
                    for k, eng in ((0, nc.sync), (1, nc.scalar)):
                        t = iop.tile([128, _F], f32, tag=tag + str(k),
                                     name=tag + str(k), bufs=3)
                        src = ysa[s + k, cs].rearrange(
                            "c i j k -> (c i) (j k)"
                        )
                        eng.dma_start(out=t[:], in_=src)
                        ts.append(t)
                    return ts

                # rev pair first (feeds the longer matmul chain), fwd
                # second.
                rC = load_pair(10, "rC")
                fC = load_pair(8, "fC")
                rB = load_pair(6, "rB")
                fB = load_pair(4, "fB")
                rA = load_pair(2, "rA")
                fA = load_pair(0, "fA")

                def pair_sums(rt, ft, tag, split_f=False):
                    rs = tmp.tile([128, _F], bf16, tag="rs" + tag,
                                  name="rs" + tag)
                    fs = tmp.tile([128, _F], bf16, tag="fs" + tag,
                                  name="fs" + tag)
                    nc.vector.tensor_add(
                        rs[:], rt[0][:][:, ::-1], rt[1][:][:, ::-1])
                    if split_f:
                        # halves pipeline into the matmul/final-add chain
                        # (shortens the end-of-kernel dependent chain).
                        for h in (slice(0, _F // 2), slice(_F // 2, _F)):
                            nc.vector.tensor_add(
                                fs[:, h], ft[0][:][:, h], ft[1][:][:, h])
                    else:
                        nc.vector.tensor_add(fs[:], ft[0][:], ft[1][:])
                    return rs, fs

                _H = (slice(0, _F // 2), slice(_F // 2, _F))

                def combine(rs, fs, name, wJx, wIx, mid=None):
                    # psum = wJx @ rs + wIx @ fs (+ optional mid member
                    # sharing wIx). Matmuls issue J,J then I,I across the
                    # two PSUM banks to minimize LDWEIGHTS switches.
                    # B and C share a tag so PSUM fits in 8 banks.
                    tag = "psA" if name == "A" else "psBC"
                    pf = ps.tile([128, _F], f32, tag=tag, name=name)
                    for h in _H:
                        nc.tensor.matmul(pf[:, h], wJx[:], rs[:][:, h],
                                         start=True, stop=False)
                    if mid is not None:
                        # both mid members before the fs members: the PE is
                        # in-order, and fs depends on the last-arriving
                        # load, so nothing may queue behind its matmuls.
                        for h in _H:
                            nc.tensor.matmul(pf[:, h], wIx[:], mid[:][:, h],
                                             start=False, stop=False)
                    for h in _H:
                        nc.tensor.matmul(pf[:, h], wIx[:], fs[:][:, h],
                                         start=False, stop=True)
                    return pf

                # C: (i,j)->(j,i) free permute as a strided READ in the
                # PSUM->SBUF copy (contiguous write), then block transpose
                # puts tCt in (j,k) layout. C runs UNSCALED (wJ1/wI1); the
                # 1/12 lands when wI accumulates tCt into psA, which lets
                # A's whole group share the wI stationary.
                rsC, fsC = pair_sums(rC, fC, "C")
                psC = combine(rsC, fsC, "C", wJ1, wI1)
                tC = tmp.tile([128, _F], bf16, tag="tC", name="tC")
                nc.scalar.copy(
                    tC[:].rearrange("p (a b) -> p a b", a=_D),
                    psC[:].rearrange("p (a b) -> p a b", a=_D).transpose(
                        [0, 2, 1]),
                )

                # B: j<->i 32x32 block transpose; leftover (k,j) free
                # permute is folded into the final add's read AP.
                rsB, fsB = pair_sums(rB, fB, "B")
                psB = combine(rsB, fsB, "B", wJ, wI)
                tB = tmp.tile([128, _F], bf16, tag="tB", name="tB")
                nc.scalar.copy(tB[:], psB[:])

                # transposes issue after all four B/C pair adds so the DVE
                # never stalls mid-chain waiting on the ACT copies.
                tCt = tmp.tile([128, _F], bf16, tag="tCt", name="tCt")
                nc.vector.transpose(tCt[:], tC[:])
                tBt = tmp.tile([128, _F], bf16, tag="tBt", name="tBt")
                nc.vector.transpose(tBt[:], tB[:])

                # A accumulates C's contribution (unscaled tCt picks up
                # its 1/12 from wI here) plus its own scans; A's fwd pair
                # is the last load, feeding only the shallow wI@fsA ->
                # final add path (no LDWEIGHTS switch left in the drain).
                rsA, fsA = pair_sums(rA, fA, "A", split_f=True)
                psA = combine(rsA, fsA, "A", wJ, wI, mid=tCt)

                # out = psA + tBt read as (j,k); f32 write, no scale op.
                # Done in 512-wide halves, each stored immediately, to
                # shorten the end-of-kernel drain. First half stores via
                # SWDGE mid-stream (keeps HWDGE rings free for loads);
                # the LAST tile uses both HWDGE rings instead - no load
                # left to displace, and HWDGE's ~0.6us first-byte latency
                # beats SWDGE's Q7 emission in the drain.
                o = tmp.tile([128, _F], f32, tag="o", name="o")
                ov = o[:].rearrange("p (j k) -> p j k", j=_D)
                pv = psA[:].rearrange("p (j k) -> p j k", j=_D)
                bv = tBt[:].rearrange("p (k j) -> p j k", k=_D)
                od = outa[cs].rearrange("c i j k -> (c i) (j k)")
                last = g == _G - 1
                for jh, eng in (
                    (slice(0, _D // 2), nc.sync if last else nc.gpsimd),
                    (slice(_D // 2, _D),
                     nc.scalar if (last or g % 2) else nc.sync),
                ):
                    nc.vector.tensor_add(ov[:, jh], pv[:, jh], bv[:, jh])
                    eng.dma_start(
                        out=od[:, _F // 2 * (jh.start != 0):][:, :_F // 2],
                        in_=o[:, _F // 2 * (jh.start != 0):][:, :_F // 2],
                    )

    nc.compile()
    return nc


def kernel(ys):
    global _nc
    ys = np.ascontiguousarray(ys, dtype=np.float32)
    assert ys.shape == (_B, _S, _C, _D, _D, _D), ys.shape

    if _nc is None:
        _nc = _build_program()

    from concourse.bass_utils import run_bass_kernel_spmd

    in_maps = []
    for r in range(_NCORE):
        b, q = divmod(r, 4)
        shard = np.ascontiguousarray(ys[b, :, q * _CL:(q + 1) * _CL])
        in_maps.append({"ys": shard})

    res = run_bass_kernel_spmd(_nc, in_maps, list(range(_NCORE)))

    out = np.empty((_B, _C, _D, _D, _D), np.float32)
    for r in range(_NCORE):
        b, q = divmod(r, 4)
        out[b, q * _CL:(q + 1) * _CL] = np.asarray(
            res.results[r]["out"]).astype(np.float32)

    if res.exec_time_ns is not None:
        print(f"HW exec time: {res.exec_time_ns} ns")
    return out



# revision 9
# speedup vs baseline: 1.1584x; 1.1503x over previous
"""Trainium2 Bass kernel for CrossMerge3D.

Input ys: [B=2, S=12, C=96, 32, 32, 32] f32. For each (b, c):
  out = (mA + perm_j(mB) + perm_k(mC)) / 12
where, with the 12 scans split into 3 groups of 4, each group combines as
  m_g = s0 + s1 + flip(s2 + s3)   (flip over the flattened 32^3 volume)
and group B's volume is stored as (j,k,i), group C's as (k,i,j).

Sharding: 8 cores = batch (2) x channel quarters (4) -> 24 channels/core.
No cross-core communication.

Per-core layout: 4 channels x 32 leading-spatial -> 128 SBUF partitions,
1024-wide free dim, 6 macro tiles. Scan-pair 1 MiB loads alternate across
both HWDGE rings; store halves split between SWDGE (gpsimd, a third
descriptor path) and the HWDGE rings. Pair sums on DVE cast to bf16
(tolerance is 2e-2; bf16 keeps ~4e-3). flip = free-dim reversal
(pair-sum read APs) + partition-block reversal, fused with the group
combine and the global 1/12 scale as accumulating bf16 matmuls against
1/12-scaled stationaries (wJ = block anti-diagonal, wI = identity).
Groups run C, B, A per tile so the deep C chain (strided copy +
transpose + extra matmul) overlaps later loads; B's leftover
(k,j)->(j,k) permute folds into the final DVE add's read AP. The final
add reads PSUM + bf16 and writes the f32 output tile in 512-wide halves
(each stored immediately) - no scale op. HW: ~119 us (baseline 145 us);
the 37.75+3.15 MB/core at the 4 KiB-descriptor line rate (~152 ns/desc
x 16 SDMA engines ~= 420 GB/s) bounds the stream at ~97 us, plus ~8 us
preamble and ~7 us drain.
"""

import numpy as np

_B, _S, _C, _D = 2, 12, 96, 32
_NCORE = 8
_CL = _C // 4          # 24 channels per core
_G = _CL // 4          # 6 macro tiles of 4 channels (128 partitions)
_F = _D * _D           # 1024

_nc = None


def _build_program():
    from concourse import bacc, tile, mybir

    f32 = mybir.dt.float32
    bf16 = mybir.dt.bfloat16
    nc = bacc.Bacc(
        "TRN2", target_bir_lowering=False, debug=False, num_devices=_NCORE
    )
    ys = nc.dram_tensor("ys", [_S, _CL, _D, _D, _D], f32, kind="ExternalInput")
    out = nc.dram_tensor("out", [_CL, _D, _D, _D], f32, kind="ExternalOutput")
    ysa = ys.ap()
    outa = out.ap()

    with tile.TileContext(nc) as tc:
        with (
            tc.tile_pool(name="const", bufs=1) as cst,
            tc.tile_pool(name="io", bufs=2) as iop,
            tc.tile_pool(name="tmp", bufs=2) as tmp,
            tc.tile_pool(name="ps", bufs=2, space="PSUM") as ps,
        ):
            scale = 1.0 / 12.0
            # stationaries: 32-block anti-diagonal (x 1/12), identity
            # (x 1/12), and an unscaled identity for the pre-scaled tCt.
            wJ = cst.tile([128, 128], bf16, tag="wJ", name="wJ")
            nc.gpsimd.memset(wJ[:], scale)
            for b in range(4):
                nc.gpsimd.affine_select(
                    out=wJ[32 * b:32 * b + 32, :],
                    in_=wJ[32 * b:32 * b + 32, :],
                    compare_op=mybir.AluOpType.is_equal, fill=0.0,
                    base=-(32 * b + 31), pattern=[[1, 128]],
                    channel_multiplier=1,
                )
            wI = cst.tile([128, 128], bf16, tag="wI", name="wI")
            nc.gpsimd.memset(wI[:], scale)
            nc.gpsimd.affine_select(
                out=wI[:], in_=wI[:],
                compare_op=mybir.AluOpType.is_equal, fill=0.0,
                base=0, pattern=[[1, 128]], channel_multiplier=-1,
            )
            wI1 = cst.tile([128, 128], bf16, tag="wI1", name="wI1")
            nc.gpsimd.memset(wI1[:], 1.0)
            nc.gpsimd.affine_select(
                out=wI1[:], in_=wI1[:],
                compare_op=mybir.AluOpType.is_equal, fill=0.0,
                base=0, pattern=[[1, 128]], channel_multiplier=-1,
            )
            wJ1 = cst.tile([128, 128], bf16, tag="wJ1", name="wJ1")
            nc.gpsimd.memset(wJ1[:], 1.0)
            for b in range(4):
                nc.gpsimd.affine_select(
                    out=wJ1[32 * b:32 * b + 32, :],
                    in_=wJ1[32 * b:32 * b + 32, :],
                    compare_op=mybir.AluOpType.is_equal, fill=0.0,
                    base=-(32 * b + 31), pattern=[[1, 128]],
                    channel_multiplier=1,
                )

            for g in range(_G):
                cs = slice(4 * g, 4 * (g + 1))

                def load_pair(s, tag):
                    # one 512 KiB DMA per scan, pair split across the two
                    # HWDGE rings: rings stay byte-balanced all the way to
                    # the drain and both pair members land together.
                    ts = []
                    for k, eng in ((0, nc.sync), (1, nc.scalar)):
                        t = iop.tile([128, _F], f32, tag=tag + str(k),
                                     name=tag + str(k), bufs=3)
                        src = ysa[s + k, cs].rearrange(
                            "c i j k -> (c i) (j k)"
                        )
                        eng.dma_start(out=t[:], in_=src)
                        ts.append(t)
                    return ts

                # rev pair first (feeds the longer matmul chain), fwd
                # second.
                rC = load_pair(10, "rC")
                fC = load_pair(8, "fC")
                rB = load_pair(6, "rB")
                fB = load_pair(4, "fB")
                rA = load_pair(2, "rA")
                fA = load_pair(0, "fA")

                def rev_sum(rt, tag):
                    rs = tmp.tile([128, _F], bf16, tag="rs" + tag,
                                  name="rs" + tag)
                    nc.vector.tensor_add(
                        rs[:], rt[0][:][:, ::-1], rt[1][:][:, ::-1])
                    return rs

                def fwd_sum(ft, tag, split=False):
                    fs = tmp.tile([128, _F], bf16, tag="fs" + tag,
                                  name="fs" + tag)
                    if split:
                        # halves pipeline into the matmul/final-add chain
                        # (shortens the end-of-kernel dependent chain).
                        for h in (slice(0, _F // 2), slice(_F // 2, _F)):
                            nc.vector.tensor_add(
                                fs[:, h], ft[0][:][:, h], ft[1][:][:, h])
                    else:
                        nc.vector.tensor_add(fs[:], ft[0][:], ft[1][:])
                    return fs

                def pair_sums(rt, ft, tag):
                    return rev_sum(rt, tag), fwd_sum(ft, tag)

                _H = (slice(0, _F // 2), slice(_F // 2, _F))

                def combine(rs, fs, name, wJx, wIx, mid=None):
                    # psum = wJx @ rs + wIx @ fs (+ optional mid member
                    # sharing wIx). Matmuls issue J,J then I,I across the
                    # two PSUM banks to minimize LDWEIGHTS switches.
                    # B and C share a tag so PSUM fits in 8 banks.
                    tag = "psA" if name == "A" else "psBC"
                    pf = ps.tile([128, _F], f32, tag=tag, name=name)
                    for h in _H:
                        nc.tensor.matmul(pf[:, h], wJx[:], rs[:][:, h],
                                         start=True, stop=False)
                    if mid is not None:
                        # both mid members before the fs members: the PE is
                        # in-order, and fs depends on the last-arriving
                        # load, so nothing may queue behind its matmuls.
                        for h in _H:
                            nc.tensor.matmul(pf[:, h], wIx[:], mid[:][:, h],
                                             start=False, stop=False)
                    for h in _H:
                        nc.tensor.matmul(pf[:, h], wIx[:], fs[:][:, h],
                                         start=False, stop=True)
                    return pf

                # C: (i,j)->(j,i) free permute as a strided READ in the
                # PSUM->SBUF copy (contiguous write), then block transpose
                # puts tCt in (j,k) layout. tCt must be bf16 (matmul
                # operand) and StreamTranspose can't cast, so C keeps the
                # ACT copy. C runs UNSCALED (wJ1/wI1); the 1/12 lands
                # when wI accumulates tCt into psA, which lets A's whole
                # group share the wI stationary.
                rsC, fsC = pair_sums(rC, fC, "C")
                psC = combine(rsC, fsC, "C", wJ1, wI1)
                tC = tmp.tile([128, _F], bf16, tag="tC", name="tC")
                nc.scalar.copy(
                    tC[:].rearrange("p (a b) -> p a b", a=_D),
                    psC[:].rearrange("p (a b) -> p a b", a=_D).transpose(
                        [0, 2, 1]),
                )

                # B: j<->i 32x32 block transpose straight out of PSUM,
                # f32 -> f32 (StreamTranspose needs matching dtypes; the
                # final add reads f32 fine). This removes the tB ACT copy
                # so the scalar engine's only compute op per tile is the
                # early-completing tC copy - its HWDGE ring can't starve
                # behind a late dependency. Leftover (k,j) free permute is
                # folded into the final add's read AP.
                rsB, fsB = pair_sums(rB, fB, "B")
                psB = combine(rsB, fsB, "B", wJ, wI)

                # A's rev sum first (rA lands before the transposes'
                # inputs are ready is fine - DVE picks what's ready), then
                # the transposes, then fsA: the DVE wait on the very last
                # load (fA) lands AFTER the transposes in queue order, so
                # they can't be dragged into the end-of-stream chain.
                rsA = rev_sum(rA, "A")
                tCt = tmp.tile([128, _F], bf16, tag="tCt", name="tCt")
                nc.vector.transpose(tCt[:], tC[:])
                tBt = tmp.tile([128, _F], f32, tag="tBt", name="tBt")
                nc.vector.transpose(tBt[:], psB[:])
                fsA = fwd_sum(fA, "A", split=True)

                # A accumulates C's contribution (unscaled tCt picks up
                # its 1/12 from wI here) plus its own scans; A's fwd pair
                # is the last load, feeding only the shallow wI@fsA ->
                # final add path (no LDWEIGHTS switch left in the drain).
                psA = combine(rsA, fsA, "A", wJ, wI, mid=tCt)

                # out = psA + tBt read as (j,k); f32 write, no scale op.
                # Done in 512-wide halves, each stored immediately, to
                # shorten the end-of-kernel drain. Mid-stream BOTH halves
                # store via SWDGE - the sync/scalar queues then carry
                # nothing but loads, so a late final add can never block
                # the next tile's load issue. The LAST tile uses both
                # HWDGE rings instead (no load left to displace, and
                # HWDGE's ~0.6us first-byte latency beats SWDGE's Q7
                # emission in the drain).
                o = tmp.tile([128, _F], f32, tag="o", name="o")
                ov = o[:].rearrange("p (j k) -> p j k", j=_D)
                pv = psA[:].rearrange("p (j k) -> p j k", j=_D)
                bv = tBt[:].rearrange("p (k j) -> p j k", k=_D)
                od = outa[cs].rearrange("c i j k -> (c i) (j k)")
                last = g == _G - 1
                for jh, eng in (
                    (slice(0, _D // 2), nc.sync if last else nc.gpsimd),
                    (slice(_D // 2, _D),
                     nc.scalar if last else nc.gpsimd),
                ):
                    nc.vector.tensor_add(ov[:, jh], pv[:, jh], bv[:, jh])
                    eng.dma_start(
                        out=od[:, _F // 2 * (jh.start != 0):][:, :_F // 2],
                        in_=o[:, _F // 2 * (jh.start != 0):][:, :_F // 2],
                    )

    nc.compile()
    return nc


def kernel(ys):
    global _nc
    ys = np.ascontiguousarray(ys, dtype=np.float32)
    assert ys.shape == (_B, _S, _C, _D, _D, _D), ys.shape

    if _nc is None:
        _nc = _build_program()

    from concourse.bass_utils import run_bass_kernel_spmd

    in_maps = []
    for r in range(_NCORE):
        b, q = divmod(r, 4)
        shard = np.ascontiguousarray(ys[b, :, q * _CL:(q + 1) * _CL])
        in_maps.append({"ys": shard})

    res = run_bass_kernel_spmd(_nc, in_maps, list(range(_NCORE)))

    out = np.empty((_B, _C, _D, _D, _D), np.float32)
    for r in range(_NCORE):
        b, q = divmod(r, 4)
        out[b, q * _CL:(q + 1) * _CL] = np.asarray(
            res.results[r]["out"]).astype(np.float32)

    if res.exec_time_ns is not None:
        print(f"HW exec time: {res.exec_time_ns} ns")
    return out

